# revision 17
# baseline (speedup 1.0000x reference)
"""AttnBlock Trainium2 Bass kernel.

Data-parallel over batch across 8 NeuronCores (4 batch elements each, full
weights on every core). Everything on-chip is feature-major ([feat, token]),
so the pipeline needs no transposes anywhere:

  x[b]                -> X   [C=256, N=1024]  (the input's natural layout)
  QK proj (PE, bf16)  -> Qst/Kst [128, 1024]  head pair stacked 64+64 rows;
                         bias folded into the DVE psum->sbuf copy
  V proj  (PE)        -> V   [N, 4*65]  [token, dim] layout; bias + a ones
                         column (at position 0 of each head block) folded in
  scores  (PE)        -> sp [128, 1024] psum: BOTH heads' K^T Q for one
                         i-half, row-tiled (tile_position (0,0)/(64,0)) so
                         the head pair streams concurrently; two sp tiles
                         ping-pong so next scores overlap the current exp
  P = exp(scale*sp)   -> ONE ACT op per (jc, ic) over [128, 1024] psum.
                         ACT is the bottleneck engine (1 elem/lane/cycle,
                         (FD+352)/1.2ns per op) => the whole kernel is paced
                         by 128 such ops; everything else hides under them.
  ctx = [1|V]^T P     -> psum [65, 1024] per head; row 0 accumulates the
                         softmax denominator Z for free
  normalize           -> ONE DVE copy frees the ctx psum slot (bf16),
                         then GPSIMD partition_broadcast of the Z row,
                         DVE reciprocal + multiply, all SBUF bf16 at 2x
  out proj (PE)       -> OUT^T [C, N] in the output layout; bias + fp32
                         residual fused in the DVE psum->sbuf pass

PSUM (16KB/partition = 8 banks, exactly full):
  sp0, sp1 [128,1024] 2 banks each — scores ping-pong; ALSO time-share as
           the psum for QK-proj halves, V-proj pairs and outproj halves,
           which are emitted as <=1.1us "fillers" one-per-jc inside the
           attention loops so the in-order PE queue never stalls on them
  c0, c1   [65,1024]  2 banks each — ctx accumulators; released 1.2us after
           each pack by the single DVE copy

Filler schedule per batch b (one filler before each jc's scores):
  pack0: V(b+1) pair0, pair1, Q1(b) half0/1, K1(b) half0/1, outproj(b-1)
         co0 half0/1
  pack1: outproj(b-1) co1 half0/1, V(b+1) pair2, pair3, Q0(b+1) half0/1,
         K0(b+1) half0/1
so next-batch Q/K are ready exactly at the batch boundary and the ACT
stream never waits on projections.

Matmul operands are bf16 (converted host-side; fp32 PSUM accumulation).
Final rel err vs the fp32 reference: ~2e-4.
"""

import numpy as np
import ml_dtypes

N_HEADS = 4
D_K = 64
SCALE = D_K ** (-0.5)
B, C, H, W = 32, 256, 32, 32
N = H * W           # 1024 tokens
NCORES = 8
BPC = B // NCORES   # 4 batch elements per core

_CACHE = {}


def _build():
    import concourse.bacc as bacc
    import concourse.mybir as mybir
    from concourse.tile import TileContext

    dt = mybir.dt
    f32 = dt.float32
    bf16 = dt.bfloat16
    EXP = mybir.ActivationFunctionType.Exp
    ADD = mybir.AluOpType.add
    MULT = mybir.AluOpType.mult

    nc = bacc.Bacc()
    x = nc.dram_tensor("x", [BPC, C, N], f32, kind="ExternalInput")
    xbf = nc.dram_tensor("xbf", [BPC, C, N], bf16, kind="ExternalInput")
    wqk = nc.dram_tensor("wqk", [C, 512], bf16, kind="ExternalInput")
    bqk = nc.dram_tensor("bqk", [128, 4], f32, kind="ExternalInput")
    wv = nc.dram_tensor("wv", [C, 260], bf16, kind="ExternalInput")
    wvb = nc.dram_tensor("wvb", [128, 260], f32, kind="ExternalInput")
    wo = nc.dram_tensor("wo", [C, C], bf16, kind="ExternalInput")
    ob = nc.dram_tensor("ob", [128, 2], f32, kind="ExternalInput")
    out = nc.dram_tensor("out", [BPC, C, N], f32, kind="ExternalOutput")

    with TileContext(nc) as tc:
        with (
            tc.tile_pool(name="consts", bufs=1) as consts,
            tc.tile_pool(name="xp", bufs=4) as xp,
            tc.tile_pool(name="qkp", bufs=4) as qkp,
            tc.tile_pool(name="vp", bufs=2) as vp,
            tc.tile_pool(name="pp", bufs=6) as pp,
            tc.tile_pool(name="miscp", bufs=4) as miscp,
            tc.tile_pool(name="outp", bufs=4) as outp,
            tc.tile_pool(name="psum", bufs=1, space="PSUM") as psum,
        ):
            # ---- load constants once (already bf16 host-side) ----
            wqk_sb = [consts.tile([128, 512], bf16, name=f"wqk{cc}") for cc in range(2)]
            wv_sb = [consts.tile([128, 260], bf16, name=f"wv{cc}") for cc in range(2)]
            wo_sb = [consts.tile([128, 256], bf16, name=f"wo{cc}") for cc in range(2)]
            bqk_sb = consts.tile([128, 4], f32, name="bqk_sb")
            wvb_sb = consts.tile([128, 260], f32, name="wvb_sb")
            ob_sb = consts.tile([128, 2], f32, name="ob_sb")
            for cc in range(2):
                nc.sync.dma_start(out=wqk_sb[cc][:], in_=wqk[cc * 128:(cc + 1) * 128, :])
                nc.sync.dma_start(out=wv_sb[cc][:], in_=wv[cc * 128:(cc + 1) * 128, :])
                nc.sync.dma_start(out=wo_sb[cc][:], in_=wo[cc * 128:(cc + 1) * 128, :])
            nc.sync.dma_start(out=bqk_sb[:], in_=bqk[:])
            nc.sync.dma_start(out=wvb_sb[:], in_=wvb[:])
            nc.sync.dma_start(out=ob_sb[:], in_=ob[:])
            warmup = consts.tile([1, 4], f32, name="warmup")
            nc.scalar.activation(warmup[:], bqk_sb[0:1, 0:4], EXP)

            xcs, xcrs, qks, vss, osbs, cns = {}, {}, {}, {}, {}, {}

            def emit_x_load(b):
                xc = [xp.tile([128, N], f32, name=f"xc{cc}", tag=f"xc{cc}")
                      for cc in range(2)]
                xcr = [xp.tile([128, N], bf16, name=f"xcr{cc}", tag=f"xcr{cc}")
                       for cc in range(2)]
                for cc in range(2):
                    nc.sync.dma_start(out=xc[cc][:], in_=x[b, cc * 128:(cc + 1) * 128, :])
                    nc.sync.dma_start(out=xcr[cc][:], in_=xbf[b, cc * 128:(cc + 1) * 128, :])
                xcs[b] = xc
                xcrs[b] = xcr

            def emit_qk_half(b, p, qk, fc):
                # one fc-half of one Q/K projection tile; psum borrows the
                # sp slots so it never waits on the normalize chain
                if b not in qks:
                    qks[b] = [[None, None], [None, None]]
                xcr = xcrs[b]
                qkps = psum.tile([128, 512], f32, name="qkps", tag=f"sp{qk}")
                col0 = p * 256 + qk * 128
                fs = slice(fc * 512, (fc + 1) * 512)
                for cc in range(2):
                    nc.tensor.matmul(
                        qkps[:],
                        wqk_sb[cc][:, col0:col0 + 128],
                        xcr[cc][:, fs],
                        start=(cc == 0), stop=(cc == 1),
                    )
                if fc == 0:
                    qks[b][p][qk] = qkp.tile([128, N], bf16, name=f"qk{p}{qk}")
                nc.vector.tensor_scalar(
                    qks[b][p][qk][:, fs], qkps[:],
                    bqk_sb[:, 2 * p + qk:2 * p + qk + 1],
                    None, ADD,
                )

            def emit_v_pair(b, pr):
                # two 128-token V chunks through one psum tile + one DVE op
                if b not in vss:
                    vss[b] = vp.tile([128, 8, 260], bf16, name="v_sb", tag="v")
                xcr = xcrs[b]
                # [128, 2, 512] so each chunk's matmul output starts on a
                # psum bank boundary; only [:, :, 0:260] is used
                vps = psum.tile([128, 2, 512], f32, name="vps", tag=f"sp{pr % 2}")
                for k in range(2):
                    jt = 2 * pr + k
                    js = slice(jt * 128, (jt + 1) * 128)
                    for cc in range(2):
                        nc.tensor.matmul(
                            vps[:, k, 0:260],
                            xcr[cc][:, js], wv_sb[cc][:],
                            start=(cc == 0), stop=(cc == 1),
                        )
                nc.vector.scalar_tensor_tensor(
                    vss[b][:, 2 * pr:2 * pr + 2, :], vps[:, :, 0:260],
                    1.0, wvb2_sb[:], MULT, ADD,
                )

            def emit_outproj_half(b, co, fc):
                if (b, co) not in osbs:
                    osbs[(b, co)] = outp.tile([128, N], f32, name="osb")
                osb = osbs[(b, co)]
                ctxn = cns[b]
                fs = slice(fc * 512, (fc + 1) * 512)
                ops = psum.tile([128, 512], f32, name="ops", tag=f"sp{co}")
                for kc in range(2):
                    nc.tensor.matmul(
                        ops[:],
                        wo_sb[kc][:, co * 128:(co + 1) * 128],
                        ctxn[kc][:, fs],
                        start=(kc == 0), stop=(kc == 1),
                    )
                nc.vector.scalar_tensor_tensor(
                    osb[:, fs], ops[:], ob_sb[:, co:co + 1], xcs[b][co][:, fs],
                    ADD, ADD,
                )
                if fc == 1:
                    nc.sync.dma_start(
                        out=out[b, co * 128:(co + 1) * 128, :], in_=osb[:]
                    )

            def emit_pack(b, p, fillers):
                # fillers: per-jc emission of <=1.1us of projection work that
                # slots into the PE/DVE slack under the ACT-paced stream
                qst, kst = qks[b][p][0], qks[b][p][1]
                v_sb = vss[b]
                ctxps = [
                    psum.tile([65, N], f32, name=f"ctx{hl}", tag=f"c{hl}")
                    for hl in range(2)
                ]
                for jc in range(8):
                    if fillers[jc] is not None:
                        fillers[jc]()
                    js = slice(jc * 128, (jc + 1) * 128)
                    for ic in range(2):
                        isl = slice(ic * 512, (ic + 1) * 512)
                        sp = psum.tile([128, N], f32, name="sp", tag=f"sp{ic}")
                        for hl in range(2):
                            hs = slice(hl * 64, (hl + 1) * 64)
                            nc.tensor.matmul(
                                sp[:, hl * 512:(hl + 1) * 512],
                                kst[hs, js],
                                qst[hs, isl],
                                start=True, stop=True,
                                tile_position=(hl * 64, 0),
                            )
                        pt = pp.tile([128, N], bf16, name="pt", tag="pt")
                        nc.scalar.activation(pt[:], sp[:], EXP, scale=SCALE)
                        for hl in range(2):
                            h = 2 * p + hl
                            nc.tensor.matmul(
                                ctxps[hl][:, isl],
                                v_sb[:, jc, h * 65:(h + 1) * 65],
                                pt[:, hl * 512:(hl + 1) * 512],
                                start=(jc == 0), stop=(jc == 7),
                            )
                # normalize: one copy releases the ctx psum slot; Z sits at
                # row 0 (ones column is FIRST in each V block), so the
                # broadcast source is partition 0; everything after is
                # SBUF bf16 at DVE 2x rate
                cn = miscp.tile([128, N], bf16, name=f"ctxn{p}", tag="cn")
                for hl in range(2):
                    cu = miscp.tile([64, N], f32, name="cu", tag="cu", bufs=3)
                    nc.vector.tensor_copy(cu[:], ctxps[hl][0:64, :])
                    z_sb = miscp.tile([1, N], f32, name="z_sb", tag="z", bufs=3)
                    nc.vector.tensor_copy(z_sb[:], ctxps[hl][64:65, :])
                    zb = miscp.tile([64, N], f32, name="zb", tag="zb", bufs=2)
                    nc.gpsimd.partition_broadcast(zb[:], z_sb[0:1, :])
                    rzb = miscp.tile([64, N], f32, name="rzb", tag="rzb", bufs=2)
                    nc.vector.reciprocal_approx_fast(rzb[:], zb[:])
                    nc.vector.tensor_tensor(
                        cn[hl * 64:(hl + 1) * 64, :],
                        cu[0:64, :],
                        rzb[:],
                        MULT,
                    )
                return cn

            wvb2_sb = consts.tile([128, 520], f32, name="wvb2_sb")
            nc.sync.dma_start(out=wvb2_sb[:, 0:260], in_=wvb[:])
            nc.sync.dma_start(out=wvb2_sb[:, 260:520], in_=wvb[:])

            # prologue: batch 0's pack-0 Q/K + all of V, plus x loads
            emit_x_load(0)
            for qk in range(2):
                for fc in range(2):
                    emit_qk_half(0, 0, qk, fc)
            for pr in range(4):
                emit_v_pair(0, pr)
            emit_x_load(1)

            def sched(b):
                """filler lists for pack0 / pack1 of batch b"""
                nxt = b + 1 if b + 1 < BPC else None
                prv = b - 1 if b >= 1 else None
                f0 = [
                    (lambda: emit_v_pair(nxt, 0)) if nxt is not None else None,
                    (lambda: emit_v_pair(nxt, 1)) if nxt is not None else None,
                    (lambda: emit_qk_half(b, 1, 0, 0)),
                    (lambda: emit_qk_half(b, 1, 0, 1)),
                    (lambda: emit_qk_half(b, 1, 1, 0)),
                    (lambda: emit_qk_half(b, 1, 1, 1)),
                    (lambda: emit_outproj_half(prv, 0, 0)) if prv is not None else None,
                    (lambda: emit_outproj_half(prv, 0, 1)) if prv is not None else None,
                ]
                f1 = [
                    (lambda: emit_outproj_half(prv, 1, 0)) if prv is not None else None,
                    (lambda: emit_outproj_half(prv, 1, 1)) if prv is not None else None,
                    (lambda: emit_v_pair(nxt, 2)) if nxt is not None else None,
                    (lambda: emit_v_pair(nxt, 3)) if nxt is not None else None,
                    (lambda: emit_qk_half(nxt, 0, 0, 0)) if nxt is not None else None,
                    (lambda: emit_qk_half(nxt, 0, 0, 1)) if nxt is not None else None,
                    (lambda: emit_qk_half(nxt, 0, 1, 0)) if nxt is not None else None,
                    (lambda: emit_qk_half(nxt, 0, 1, 1)) if nxt is not None else None,
                ]
                return f0, f1

            for b in range(BPC):
                f0, f1 = sched(b)
                cn0 = emit_pack(b, 0, f0)
                cn1 = emit_pack(b, 1, f1)
                cns[b] = [cn0, cn1]
                if b + 2 < BPC:
                    emit_x_load(b + 2)
            # tail: last batch's out projection
            for co in range(2):
                for fc in range(2):
                    emit_outproj_half(BPC - 1, co, fc)

    nc.compile()
    return nc


def _prep_weights(proj_w, proj_b, out_w, out_b):
    qk_cols = []
    for p in range(2):
        for qk in range(2):
            for hl in range(2):
                h = 2 * p + hl
                base = h * 192 + qk * 64
                qk_cols.extend(range(base, base + 64))
    wqk = np.ascontiguousarray(proj_w[qk_cols, :].T).astype(ml_dtypes.bfloat16)
    bqk = np.ascontiguousarray(
        proj_b[qk_cols].reshape(4, 128).T                     # [128, 4]
    )

    wv = np.zeros((C, 260), dtype=np.float32)
    wvb1 = np.zeros((1, 260), dtype=np.float32)
    for h in range(N_HEADS):
        rows = range(h * 192 + 128, h * 192 + 192)
        wv[:, h * 65:h * 65 + 64] = proj_w[rows, :].T
        wvb1[0, h * 65:h * 65 + 64] = proj_b[rows]
        wvb1[0, h * 65 + 64] = 1.0
    wvb = np.ascontiguousarray(np.repeat(wvb1, 128, axis=0))  # [128, 260]
    wv = wv.astype(ml_dtypes.bfloat16)

    wo = np.ascontiguousarray(out_w.T).astype(ml_dtypes.bfloat16)
    ob = np.ascontiguousarray(out_b.reshape(2, 128).T)        # [128, 2]
    return dict(wqk=wqk, bqk=bqk, wv=wv, wvb=wvb, wo=wo, ob=ob)


def kernel(x, proj_w, proj_b, out_w, out_b, _trace=False):
    from concourse.bass_utils import run_bass_kernel_spmd

    x = np.asarray(x, dtype=np.float32)
    proj_w = np.asarray(proj_w, dtype=np.float32)
    proj_b = np.asarray(proj_b, dtype=np.float32)
    out_w = np.asarray(out_w, dtype=np.float32)
    out_b = np.asarray(out_b, dtype=np.float32)

    if "nc" not in _CACHE:
        _CACHE["nc"] = _build()
    nc = _CACHE["nc"]

    w = _prep_weights(proj_w, proj_b, out_w, out_b)
    xs = np.ascontiguousarray(x.reshape(B, C, N))
    xsbf = xs.astype(ml_dtypes.bfloat16)
    in_maps = [
        dict(w, x=np.ascontiguousarray(xs[i * BPC:(i + 1) * BPC]),
             xbf=np.ascontiguousarray(xsbf[i * BPC:(i + 1) * BPC]))
        for i in range(NCORES)
    ]
    res = run_bass_kernel_spmd(nc, in_maps, core_ids=list(range(NCORES)), trace=_trace)
    out = np.concatenate([r["out"] for r in res.results], axis=0)
    out = out.reshape(B, C, H, W)
    if _trace:
        _CACHE["last_result"] = res
    return out
